# revision 7
# baseline (speedup 1.0000x reference)
"""Trainium2 Bass kernel for nn_GateActivation (e3nn gate: 512x0e + 256x1o + 128x2e).

Strategy:
  - Data-parallel over rows: 65536 rows -> 8 cores x 8192 rows; weights replicated.
  - Host transposes each shard to feature-major [1920, R] with the l>0 irreps
    de-interleaved to plane-major order, so every on-chip matmul is a plain
    weights-stationary `out = W.T @ actsT` with no on-chip transposes at all.
  - All matmuls run as float32r (reduced-precision fp32, full PE rate at N>=256).
  - Gate activations use only the Sigmoid ACT table (silu = x * sigmoid(x) via a
    DVE multiply), so the ~2.7us table swap is paid once.
  - Feature-major layout makes the gate broadcast partition-aligned: a single
    elementwise tensor_mul per 128-feature chunk, no gate replication.
"""

import os
import sys
from contextlib import ExitStack

import numpy as np

sys.path.insert(0, "/opt/trn_rl_repo")

import concourse.bass as bass  # noqa: E402
import concourse.tile as tile  # noqa: E402
from concourse import bacc, mybir  # noqa: E402
from concourse.bass_utils import run_bass_kernel_spmd  # noqa: E402

# Problem shape (hardcoded per harness contract)
N_ROWS = 65536
N_CORES = 8
R = N_ROWS // N_CORES  # rows per core
D_IN = 1920
M0, M1, M2 = 512, 256, 128
GRP = 512  # rows per on-chip group (matmul moving free dim)
NCHUNK = D_IN // 128  # 15 feature chunks of 128

F32 = mybir.dt.float32
F32R = mybir.dt.float32r
SIGMOID = mybir.ActivationFunctionType.Sigmoid

USE_F32R = os.environ.get("BASS_GATE_FP32_FULL", "0") != "1"
MDT = F32R if USE_F32R else F32  # dtype for anything feeding a matmul

# Stash of the last run's results for test harness introspection
last_results = None


def build_nc(rows=R, grp=GRP):
    """Build the per-core Bass program (SPMD; same program on all 8 cores)."""
    assert rows % grp == 0
    n_groups = rows // grp
    nc = bacc.Bacc("TRN2", target_bir_lowering=False, debug=False)

    xT = nc.dram_tensor("xT", [n_groups, 128, NCHUNK * grp], MDT, kind="ExternalInput")
    wall = nc.dram_tensor("wall", [128, 6912], MDT, kind="ExternalInput")
    outT = nc.dram_tensor("outT", [n_groups, 128, NCHUNK * grp], F32, kind="ExternalOutput")

    with TileKernel(nc) as tk:
        tk.emit(xT, wall, outT, n_groups, grp, rows)
    nc.compile()
    return nc


class TileKernel:
    def __init__(self, nc):
        self.nc = nc
        self.ctx = ExitStack()

    def __enter__(self):
        self.tc = self.ctx.enter_context(tile.TileContext(self.nc))
        return self

    def __exit__(self, *exc):
        return self.ctx.__exit__(*exc)

    def emit(self, xT, wall, outT, n_groups, grp, rows):
        nc, tc, ctx = self.nc, self.tc, self.ctx

        wpool = ctx.enter_context(tc.tile_pool(name="w", bufs=1))
        xpool = ctx.enter_context(tc.tile_pool(name="x", bufs=2))
        ypool = ctx.enter_context(tc.tile_pool(name="y", bufs=2))
        apool = ctx.enter_context(tc.tile_pool(name="act", bufs=18))
        pre_ps = ctx.enter_context(
            tc.tile_pool(name="pre_ps", bufs=4, space=bass.MemorySpace.PSUM))
        post_ps = ctx.enter_context(
            tc.tile_pool(name="post_ps", bufs=4, space=bass.MemorySpace.PSUM))

        # --- load all weights in one DMA (resident for the whole kernel) ---
        wt = wpool.tile([128, 6912], MDT, tag="wall")
        nc.gpsimd.dma_start(wt[:], wall[:])

        def w0pre_s(k, m):
            return wt[:, k * 896 + m * 128:k * 896 + (m + 1) * 128]

        def w1pre_s(k, c):
            return wt[:, 3584 + k * 256 + c * 128:3584 + k * 256 + (c + 1) * 128]

        w2pre_s = wt[:, 4096:4224]

        def w0post_s(k, m):
            return wt[:, 4224 + k * 512 + m * 128:4224 + k * 512 + (m + 1) * 128]

        def w1post_s(k, c):
            return wt[:, 6272 + k * 256 + c * 128:6272 + k * 256 + (c + 1) * 128]

        w2post_s = wt[:, 6784:6912]

        for g in range(n_groups):
            xt_s = xpool.tile([128, 4 * grp], MDT, tag="xt_s")
            nc.sync.dma_start(xt_s[:], xT[g, :, :4 * grp])
            xt_v1 = xpool.tile([128, 6 * grp], MDT, tag="xt_v1")
            nc.sync.dma_start(xt_v1[:], xT[g, :, 4 * grp:10 * grp])
            xt_v2 = xpool.tile([128, 5 * grp], MDT, tag="xt_v2")
            nc.sync.dma_start(xt_v2[:], xT[g, :, 10 * grp:])

            def xtc(c):
                if c < 4:
                    return xt_s[:, c * grp:(c + 1) * grp]
                if c < 10:
                    return xt_v1[:, (c - 4) * grp:(c - 3) * grp]
                return xt_v2[:, (c - 10) * grp:(c - 9) * grp]

            # --- pre-gate scalar path: s_preT chunks m=0..6 ---
            # m in 0..3 -> silu chunks (sc), m in 4..6 -> gate chunks (sigmoid)
            # Emit gate chunks first so gates are ready when v-planes arrive.
            sc = [None] * 4
            gt = [None] * 3
            for m in (4, 5, 6, 0, 1, 2, 3):
                ps = pre_ps.tile([128, grp], F32, tag="pre")
                for k in range(4):
                    nc.tensor.matmul(
                        ps[:],
                        w0pre_s(k, m),
                        xtc(k),
                        start=(k == 0), stop=(k == 3))
                if m >= 4:
                    gch = apool.tile([128, grp], F32, tag="act")
                    nc.scalar.activation(gch[:], ps[:], SIGMOID)
                    gt[m - 4] = gch
                else:
                    sg = apool.tile([128, grp], F32, tag="act")
                    nc.scalar.activation(sg[:], ps[:], SIGMOID)
                    sch = apool.tile([128, grp], MDT, tag="act")
                    nc.vector.tensor_mul(sch[:], ps[:], sg[:])  # silu = x*sig(x)
                    sc[m] = sch

            # --- pre-gate v1 (3 planes x 2 v-chunks) + gating ---
            v1g = [[None] * 2 for _ in range(3)]
            for i in range(3):
                for c in range(2):
                    ps = pre_ps.tile([128, grp], F32, tag="pre")
                    for k in range(2):
                        nc.tensor.matmul(
                            ps[:],
                            w1pre_s(k, c),
                            xtc(4 + 2 * i + k),
                            start=(k == 0), stop=(k == 1))
                    vg = apool.tile([128, grp], MDT, tag="act")
                    nc.vector.tensor_mul(vg[:], ps[:], gt[c][:])
                    v1g[i][c] = vg

            # --- pre-gate v2 (5 planes) + gating ---
            v2g = [None] * 5
            for i in range(5):
                ps = pre_ps.tile([128, grp], F32, tag="pre")
                nc.tensor.matmul(
                    ps[:],
                    w2pre_s,
                    xtc(10 + i),
                    start=True, stop=True)
                vg = apool.tile([128, grp], MDT, tag="act")
                nc.vector.tensor_mul(vg[:], ps[:], gt[2][:])
                v2g[i] = vg

            # --- post-gate ---
            yt_a = ypool.tile([128, 8 * grp], F32, tag="yt_a")
            yt_b = ypool.tile([128, 7 * grp], F32, tag="yt_b")

            def ytc(chunk):
                if chunk < 8:
                    return yt_a[:, chunk * grp:(chunk + 1) * grp]
                return yt_b[:, (chunk - 8) * grp:(chunk - 7) * grp]

            def evac(ps, chunk, on_dve):
                if on_dve:
                    nc.vector.tensor_copy(ytc(chunk), ps[:])
                else:
                    nc.scalar.copy(ytc(chunk), ps[:])
                if chunk == 7:
                    nc.scalar.dma_start(outT[g, :, :8 * grp], yt_a[:])
                elif chunk == 14:
                    nc.scalar.dma_start(outT[g, :, 8 * grp:], yt_b[:])

            for m in range(4):  # scalar out chunks
                ps = post_ps.tile([128, grp], F32, tag="post")
                for k in range(4):
                    nc.tensor.matmul(
                        ps[:],
                        w0post_s(k, m),
                        sc[k][:],
                        start=(k == 0), stop=(k == 3))
                evac(ps, m, on_dve=False)

            for i in range(3):  # v1 out planes
                for c in range(2):
                    ps = post_ps.tile([128, grp], F32, tag="post")
                    for k in range(2):
                        nc.tensor.matmul(
                            ps[:],
                            w1post_s(k, c),
                            v1g[i][k][:],
                            start=(k == 0), stop=(k == 1))
                    evac(ps, 4 + 2 * i + c, on_dve=False)

            for i in range(5):  # v2 out planes
                ps = post_ps.tile([128, grp], F32, tag="post")
                nc.tensor.matmul(
                    ps[:],
                    w2post_s,
                    v2g[i][:],
                    start=True, stop=True)
                evac(ps, 10 + i, on_dve=(i >= 2))



# ---------------------------------------------------------------------------
# Host-side layout transforms
# ---------------------------------------------------------------------------

def to_feature_major(xs):
    """[r, 1920] row-major -> [1920, r] feature-major, v1/v2 plane-major rows."""
    r = xs.shape[0]
    xT = np.empty((D_IN, r), np.float32)
    xT[:M0] = xs[:, :M0].T
    xT[M0:M0 + 3 * M1] = (
        xs[:, M0:M0 + 3 * M1].reshape(r, M1, 3).transpose(2, 1, 0).reshape(3 * M1, r))
    xT[M0 + 3 * M1:] = (
        xs[:, M0 + 3 * M1:].reshape(r, M2, 5).transpose(2, 1, 0).reshape(5 * M2, r))
    return xT


def from_feature_major(yT):
    """Inverse of to_feature_major."""
    r = yT.shape[1]
    out = np.empty((r, D_IN), np.float32)
    out[:, :M0] = yT[:M0].T
    out[:, M0:M0 + 3 * M1] = (
        yT[M0:M0 + 3 * M1].reshape(3, M1, r).transpose(2, 1, 0).reshape(r, 3 * M1))
    out[:, M0 + 3 * M1:] = (
        yT[M0 + 3 * M1:].reshape(5, M2, r).transpose(2, 1, 0).reshape(r, 5 * M2))
    return out


def prep_weights(W0_pre, W1_pre, W2_pre, W0_post, W1_post, W2_post):
    f = np.float32

    def chunks(w, scale, kchunks):
        # [K, M] -> [128, kchunks*M]: partition p holds rows {k*128+p}
        return (w / np.sqrt(scale)).astype(f).reshape(
            kchunks, 128, -1).transpose(1, 0, 2).reshape(128, -1)

    wall = np.concatenate([
        chunks(W0_pre, M0, 4), chunks(W1_pre, M1, 2), chunks(W2_pre, M2, 1),
        chunks(W0_post, M0, 4), chunks(W1_post, M1, 2), chunks(W2_post, M2, 1),
    ], axis=1)
    assert wall.shape == (128, 6912), wall.shape
    return {"wall": np.ascontiguousarray(wall)}


def to_groups(xT, grp=GRP):
    """[1920, r] -> [G, 128, 15*grp] partition-major per-group blocks."""
    r = xT.shape[1]
    g = r // grp
    return np.ascontiguousarray(
        xT.reshape(NCHUNK, 128, g, grp).transpose(2, 1, 0, 3)).reshape(
            g, 128, NCHUNK * grp)


def from_groups(xTg):
    """[G, 128, 15*grp] -> [1920, r]."""
    g = xTg.shape[0]
    grp = xTg.shape[2] // NCHUNK
    return np.ascontiguousarray(
        xTg.reshape(g, 128, NCHUNK, grp).transpose(2, 1, 0, 3)).reshape(
            D_IN, g * grp)


_nc_cache = {}


def _get_nc(rows=R):
    key = (rows, USE_F32R)
    if key not in _nc_cache:
        _nc_cache[key] = build_nc(rows=rows)
    return _nc_cache[key]


def kernel(x, W0_pre, W1_pre, W2_pre, W0_post, W1_post, W2_post):
    global last_results
    x = np.asarray(x, dtype=np.float32)
    assert x.shape == (N_ROWS, D_IN), x.shape

    wmaps = prep_weights(
        np.asarray(W0_pre), np.asarray(W1_pre), np.asarray(W2_pre),
        np.asarray(W0_post), np.asarray(W1_post), np.asarray(W2_post))

    nc = _get_nc()
    in_maps = []
    for c in range(N_CORES):
        shard = x[c * R:(c + 1) * R]
        m = {"xT": to_groups(to_feature_major(shard))}
        m.update(wmaps)
        in_maps.append(m)

    trace = os.environ.get("BASS_GATE_TRACE", "0") == "1"
    last_results = run_bass_kernel_spmd(
        nc, in_maps, list(range(N_CORES)), trace=trace)

    out = np.empty((N_ROWS, D_IN), np.float32)
    for c in range(N_CORES):
        out[c * R:(c + 1) * R] = from_feature_major(
            from_groups(last_results.results[c]["outT"]))
    return out
